# revision 18
# baseline (speedup 1.0000x reference)
"""DeepPoly ReLU abstract-transformer kernel for 8 TRN2 NeuronCores.

Reference semantics (elementwise over N = 16,777,216):
    x_out     = relu(x)
    neg  = upper <= 0          -> bounds (0, 0)
    pos  = lower >= 0          -> bounds (upper, upper)
    crossing   (else)          -> (lower, upper^2 / (upper - lower))

Memory-bound elementwise problem, harness tolerance rel_err < 2e-2
(max-abs / global-max).  HBM traffic is cut from 24 B/elem (f32) to
9 B/elem with mixed-precision I/O:

  x         int8  (linear quant, q = round(x/sx), sx = max|x|/127) -> x_out int8
  upper_out int8  (scale su, folded into the Ln/Exp pipeline for free)
  l, u      bf16  (branch decisions need exact signs; lower_out carries l)
  lower_out bf16

Why this split is safe: x_out = relu(x) and upper_out are *continuous* in
the inputs, so linear-quantization error stays bounded by ~max/254 -> ~4e-3
of the global max.  l and u must keep exact signs: lower_out is
discontinuous across the l>=0 and u<=0 branch boundaries, so quantizing
them near 0 would produce O(max) errors; bf16 preserves signs exactly.

Device math (bass blocks ACT Reciprocal; reciprocal_approx_* is f32-only and
would push DVE into slow 1x-mode ops, so the division runs in log space on
the otherwise-idle ACT engine -- natural_log_exp_and_others is ONE table
set, loaded once; nothing else may touch ACT or it thrashes table reloads):
    up  = max(u, 1e-19)              DVE   (guards Ln(0); error ~1e-19)
    mn  = min(l, 0)                  DVE
    d   = up - mn                    DVE   [= relu(u)+relu(-l) >= 1e-19]
    l1  = Ln(up * 1/sqrt(su)) f16    ACT   (su folded in via [P,1] AP scale)
    l2  = Ln(d)               f16    ACT
    c   = 2*l1 - l2           f16    DVE scalar_tensor_tensor
    uq  = Exp(c) -> int8             ACT   [= up^2/(su*d) in [0,127]]
    t   = max(-u, l)                 DVE   (t < 0  <=>  crossing, exact)
    m   = (t < 0) as u16             DVE   (u16: keeps 16-bit 2x DVE mode)
    lower_out: up, then copy_predicated(m) <- l    DVE
    x_out q: max(qx, 0)              DVE   (int domain, exact)
Host dequantizes x_out = sx*q, upper_out = su*q.  All DVE operands are
16-bit where possible -- sustained throughput is DVE-op-bound, and 8-bit
operands drop DVE to 1x mode.

Sharding: pure elementwise -> contiguous 1/8 slice of N per core, no
communication.  Per core the slice is [ntiles, P=128, T]; l/u are packed
into one [ntiles, P, 2T] bf16 DRAM tensor and x_out/upper_out into one
[ntiles, P, 2T] int8 tensor, so each iteration is 2 input DMAs + 2 output
DMAs of contiguous per-(tile,partition) chunks.  reps>1 wraps the body in a
hardware For_i loop (benchmarking only; staggered_reset is available via
stagger=1 and saves ~2us/rep of back-edge barrier, but is off by default --
one validation run of it hit a device-unrecoverable fault).
"""

import numpy as np

import concourse.bacc as bacc
import concourse.mybir as mybir
import concourse.tile as tile
from concourse import bass_utils
from concourse.alu_op_type import AluOpType

N_CORES = 8
N_TOTAL = 16777216
P = 128
PER_CORE = N_TOTAL // N_CORES          # 2,097,152
NCOLS = PER_CORE // P                  # 16,384 columns per core

TILE_F = 2048
BUFS = 3
IN_RING = "sync"
OUT_RING = "scalar"

_BF = mybir.dt.bfloat16
_F16 = mybir.dt.float16
_F32 = mybir.dt.float32
_U16 = mybir.dt.uint16
_I8 = mybir.dt.int8
_NP_BF = mybir.dt.np(_BF)
_LN = mybir.ActivationFunctionType.Ln
_EXP = mybir.ActivationFunctionType.Exp


def build_nc(
    tile_f: int = TILE_F,
    bufs: int = BUFS,
    reps: int = 1,
    in_ring: str = IN_RING,
    out_ring: str = OUT_RING,
    stagger: int = 0,
):
    assert NCOLS % tile_f == 0
    ntiles = NCOLS // tile_f
    t = tile_f
    nc = bacc.Bacc(
        "TRN2", target_bir_lowering=False, debug=False, num_devices=N_CORES
    )
    xin = nc.dram_tensor("xin", [ntiles, P, t], _I8, kind="ExternalInput").ap()
    lu = nc.dram_tensor("lu", [ntiles, P, 2 * t], _BF, kind="ExternalInput").ap()
    sc = nc.dram_tensor("sc", [P, 1], _F32, kind="ExternalInput").ap()
    xuo = nc.dram_tensor(
        "xuo", [ntiles, P, 2 * t], _I8, kind="ExternalOutput"
    ).ap()
    loo = nc.dram_tensor("loo", [ntiles, P, t], _BF, kind="ExternalOutput").ap()

    with tile.TileContext(nc) as tc:
        with (
            tc.tile_pool(name="const", bufs=1) as cpool,
            tc.tile_pool(name="io", bufs=bufs) as pool,
        ):
            sct = cpool.tile([P, 1], _F32, tag="sc")
            nc.sync.dma_start(out=sct[:], in_=sc)

            def one_iter(i):
                xt = pool.tile([P, t], _I8, tag="x")
                getattr(nc, in_ring).dma_start(out=xt[:], in_=xin[i])
                lt = pool.tile([P, 2 * t], _BF, tag="lu")
                getattr(nc, in_ring).dma_start(out=lt[:], in_=lu[i])
                ls = lt[:, 0:t]
                us = lt[:, t : 2 * t]

                xu = pool.tile([P, 2 * t], _I8, tag="xu")
                xq = xu[:, 0:t]
                uq = xu[:, t : 2 * t]
                lo = pool.tile([P, t], _BF, tag="lo")

                nc.vector.tensor_scalar(
                    out=xq, in0=xt[:], scalar1=0, scalar2=None,
                    op0=AluOpType.max,
                )
                # up = max(u, 1e-19): neg lanes ~0, pos lanes u; doubles as
                # the lower_out base (crossing lanes patched below).
                nc.vector.tensor_scalar(
                    out=lo[:], in0=us, scalar1=1e-19, scalar2=None,
                    op0=AluOpType.max,
                )
                dn = pool.tile([P, t], _BF, tag="d")
                nc.vector.tensor_scalar(
                    out=dn[:], in0=ls, scalar1=0.0, scalar2=None,
                    op0=AluOpType.min,
                )
                nc.vector.tensor_sub(out=dn[:], in0=lo[:], in1=dn[:])  # d

                l1 = pool.tile([P, t], _F16, tag="lnu")
                nc.scalar.activation(l1[:], lo[:], _LN, scale=sct[:])
                l2 = pool.tile([P, t], _F16, tag="lnd")
                nc.scalar.activation(l2[:], dn[:], _LN)
                nc.vector.scalar_tensor_tensor(
                    out=l2[:], in0=l1[:], scalar=2.0, in1=l2[:],
                    op0=AluOpType.mult, op1=AluOpType.subtract,
                )
                nc.scalar.activation(uq, l2[:], _EXP)

                # crossing mask: t = max(-u, l) < 0  <=>  (u > 0) & (l < 0),
                # an exact sign test on the bf16 inputs.
                tt = pool.tile([P, t], _BF, tag="t")
                nc.vector.scalar_tensor_tensor(
                    out=tt[:], in0=us, scalar=-1.0, in1=ls,
                    op0=AluOpType.mult, op1=AluOpType.max,
                )
                mm = pool.tile([P, t], _U16, tag="m")
                nc.vector.tensor_scalar(
                    out=mm[:], in0=tt[:], scalar1=0.0, scalar2=None,
                    op0=AluOpType.is_lt,
                )
                nc.vector.copy_predicated(out=lo[:], mask=mm[:], data=ls)

                getattr(nc, out_ring).dma_start(out=xuo[i], in_=xu[:])
                getattr(nc, out_ring).dma_start(out=loo[i], in_=lo[:])

            def body():
                for i in range(ntiles):
                    one_iter(i)

            if reps == 1:
                body()
            else:
                with tc.For_i(0, reps, 1, staggered_reset=bool(stagger)):
                    body()
    nc.compile()
    return nc


def _scales(inputs):
    x = np.asarray(inputs["x"], dtype=np.float32)
    u = np.asarray(inputs["upper"], dtype=np.float32)
    sx = float(np.abs(x).max()) / 127.0
    u_bf = u.astype(_NP_BF).astype(np.float32)
    su = float(np.maximum(u_bf, 0.0).max()) * 1.01 / 127.0
    return sx, su


def prep_inputs(inputs: dict, tile_f: int = TILE_F) -> dict:
    """FULL f32 inputs -> per-core device tensors + host-side scales."""
    t = tile_f
    ntiles = NCOLS // t
    sx, su = _scales(inputs)
    x = np.asarray(inputs["x"], dtype=np.float32)
    qx = np.clip(np.round(x / sx), -127, 127).astype(np.int8)
    xin = qx.reshape(N_CORES, ntiles, P, t)
    lu = np.empty((N_CORES, ntiles, P, 2 * t), dtype=_NP_BF)
    for j, k in enumerate(("lower", "upper")):
        a = np.asarray(inputs[k], dtype=np.float32).reshape(
            N_CORES, ntiles, P, t
        )
        lu[:, :, :, j * t : (j + 1) * t] = a.astype(_NP_BF)
    sc = np.full((N_CORES, P, 1), 1.0 / np.sqrt(su), dtype=np.float32)
    return {"xin": xin, "lu": lu, "sc": sc, "_sx": sx, "_su": su}


def unpack_outputs(outs: dict, tile_f: int = TILE_F, sx=None, su=None):
    """outs: {"xuo": [N_CORES, nt, P, 2t] i8, "loo": [N_CORES, nt, P, t] bf}"""
    t = tile_f
    xuo = outs["xuo"]
    xo = xuo[:, :, :, 0:t].astype(np.float32) * sx
    uo = xuo[:, :, :, t : 2 * t].astype(np.float32) * su
    lo = np.ascontiguousarray(outs["loo"]).astype(np.float32)
    return (
        xo.reshape(1, N_TOTAL),
        lo.reshape(1, N_TOTAL),
        uo.reshape(1, N_TOTAL),
    )


def unpack_from(outs: dict, prep: dict, tile_f: int = TILE_F):
    """Generic-harness hook: outs maps output name -> [N_CORES, ...] array."""
    return unpack_outputs(outs, tile_f=tile_f, sx=prep["_sx"], su=prep["_su"])


def run(inputs: dict, trace: bool = False):
    """Shard, execute on 8 cores, gather. Returns (outputs_tuple, results)."""
    pk = prep_inputs(inputs)
    sx, su = pk.pop("_sx"), pk.pop("_su")
    in_maps = [
        {"xin": pk["xin"][c], "lu": pk["lu"][c], "sc": pk["sc"][c]}
        for c in range(N_CORES)
    ]
    nc = build_nc()
    res = bass_utils.run_bass_kernel_spmd(
        nc, in_maps, core_ids=list(range(N_CORES)), trace=trace
    )
    outs = {
        k: np.stack([res.results[c][k] for c in range(N_CORES)])
        for k in ("xuo", "loo")
    }
    return unpack_outputs(outs, sx=sx, su=su), res


def kernel(**inputs):
    outs, _ = run(inputs, trace=False)
    return outs
